# revision 4
# baseline (speedup 1.0000x reference)
"""Distributed NT-Xent (SimCLR) contrastive loss on 8 Trainium2 NeuronCores.

Math (reference): z = l2norm(rows of [emb_i; emb_j]); sim = z @ z.T;
loss = mean_k( log(sum_{j!=k} exp(sim[k,j]/T)) - sim[k, partner(k)]/T ).

Distribution: core c owns rows emb_i[c*512:(c+1)*512] and emb_j[c*512:(c+1)*512]
(1024 local rows).  The denominator is an order-independent sum over all 8192
columns, so the gather order of the column blocks is irrelevant, and the
positive pair of every local row is also local (row r <-> row r+512).  All 8
cores therefore run an *identical* program on different data:

  1. normalize own 1024x1024 fp32 shard, cast to bf16
  2. transpose to [D, rows] via PE so D is the contraction (partition) dim
  3. AllGather the 2MB transposed shard -> 16MB [8, 1024, 1024]
  4. GEMM own rows x all 8192 gathered columns (bf16, fp32 PSUM accum),
     fused Exp(2x)+row-accumulate epilogue on the scalar engine
  5. emit per-row: rowsum (incl. self term), self-sim diag, positive sim

Host combines the 3 small per-core vectors in float64.
"""

import numpy as np
import ml_dtypes

NCORES = 8
N = 4096          # rows per emb matrix
D = 1024          # embedding dim
RLOC = 1024       # local rows per core (512 from emb_i + 512 from emb_j)
HALF = RLOC // 2  # 512
P = 128

_CACHE = {}


def _build():
    from contextlib import ExitStack
    import concourse.bacc as bacc
    import concourse.tile as tile
    import concourse.mybir as mybir

    dt = mybir.dt
    AF = mybir.ActivationFunctionType
    ALU = mybir.AluOpType
    AX = mybir.AxisListType

    nc = bacc.Bacc(
        "TRN2", target_bir_lowering=False, debug=False, num_devices=NCORES
    )

    x = nc.dram_tensor("x", [RLOC, D], dt.float32, kind="ExternalInput")
    ident = nc.dram_tensor("ident", [P, P], dt.bfloat16, kind="ExternalInput")
    rowsum_o = nc.dram_tensor("rowsum", [P, 8], dt.float32, kind="ExternalOutput")
    posv_o = nc.dram_tensor("posv", [P, 4], dt.float32, kind="ExternalOutput")
    diag_o = nc.dram_tensor("diagv", [P, 8], dt.float32, kind="ExternalOutput")

    with tile.TileContext(nc) as tc, ExitStack() as ctx:
        persist = ctx.enter_context(tc.tile_pool(name="persist", bufs=1))
        xpool = ctx.enter_context(tc.tile_pool(name="xin", bufs=3))
        junkp = ctx.enter_context(tc.tile_pool(name="junk", bufs=2))
        ptpool = ctx.enter_context(tc.tile_pool(name="pt", bufs=4, space="PSUM"))
        pspool = ctx.enter_context(tc.tile_pool(name="ps", bufs=4, space="PSUM"))
        rhspool = ctx.enter_context(tc.tile_pool(name="rhsp", bufs=3))
        scrpool = ctx.enter_context(tc.tile_pool(name="scrp", bufs=3))
        dram = ctx.enter_context(tc.tile_pool(name="dram", bufs=1, space="DRAM"))

        ident_sb = persist.tile([P, P], dt.bfloat16)
        nc.sync.dma_start(ident_sb[:], ident[:, :])

        z32 = persist.tile([P, 8 * D], dt.float32)     # normalized rows, fp32
        zb = persist.tile([P, 8 * D], dt.bfloat16)     # normalized rows, bf16
        zT = persist.tile([P, 8 * RLOC], dt.bfloat16)  # transposed: [p, k*RLOC + r] = zb[r, k*128+p]
        ssq = persist.tile([P, 8], dt.float32)
        nrm = persist.tile([P, 8], dt.float32)
        inv = persist.tile([P, 8], dt.float32)
        diag_sb = persist.tile([P, 8], dt.float32)
        pos_sb = persist.tile([P, 4], dt.float32)
        rs = persist.tile([P, 8 * 16], dt.float32)     # exp row-partials per (m, b*2+n)
        rowsum_sb = persist.tile([P, 8], dt.float32)

        # ---- Phase A: load + row-normalize own shard -------------------
        for m in range(8):
            xt = xpool.tile([P, D], dt.float32, tag="xt")
            nc.sync.dma_start(xt[:], x[m * P:(m + 1) * P, :])
            sj = junkp.tile([P, D], dt.float32, tag="sj")
            nc.scalar.activation(sj[:], xt[:], AF.Square,
                                 accum_out=ssq[:, m:m + 1])
            nc.scalar.sqrt(nrm[:, m:m + 1], ssq[:, m:m + 1])
            nc.vector.reciprocal(inv[:, m:m + 1], nrm[:, m:m + 1])
            nc.vector.tensor_scalar_mul(z32[:, m * D:(m + 1) * D], xt[:],
                                        inv[:, m:m + 1])
            nc.vector.tensor_copy(zb[:, m * D:(m + 1) * D],
                                  z32[:, m * D:(m + 1) * D])
            sjb = junkp.tile([P, D], dt.bfloat16, tag="sjb")
            nc.scalar.activation(sjb[:], zb[:, m * D:(m + 1) * D], AF.Square,
                                 accum_out=diag_sb[:, m:m + 1])

        # positives: local row r pairs with local row r+512 (fp32 dot)
        # (tensor_tensor_reduce crashes the axon terminal; use mul+reduce)
        for t in range(4):
            pj = junkp.tile([P, D], dt.float32, tag="sj")
            nc.vector.tensor_mul(pj[:], z32[:, t * D:(t + 1) * D],
                                 z32[:, (t + 4) * D:(t + 5) * D])
            nc.vector.reduce_sum(pos_sb[:, t:t + 1], pj[:], axis=AX.X)

        # ---- Phase B: transpose zb -> zT (64 PE transposes) ------------
        for m in range(8):
            for k in range(8):
                pt = ptpool.tile([P, P], dt.bfloat16, tag="pt")
                nc.tensor.transpose(
                    pt[:], zb[:, m * D + k * P: m * D + (k + 1) * P],
                    ident_sb[:])
                nc.vector.tensor_copy(
                    zT[:, k * RLOC + m * P: k * RLOC + (m + 1) * P], pt[:])

        # ---- Phase C: bounce to DRAM + AllGather -----------------------
        import os
        no_cc = bool(os.environ.get("KERNEL_NO_CC"))
        bounce = dram.tile([D, RLOC], dt.bfloat16)
        nc.sync.dma_start(bounce.rearrange("(k p) r -> p k r", p=P),
                          zT.rearrange("p (k r) -> p k r", k=8))
        if not no_cc:
            gathered = dram.tile([NCORES, D, RLOC], dt.bfloat16,
                                 addr_space="Shared")
            nc.gpsimd.collective_compute(
                "AllGather", ALU.bypass,
                replica_groups=[list(range(NCORES))],
                ins=[bounce.opt()], outs=[gathered.opt()])

        # ---- Phase D: GEMM + fused exp/row-sum epilogue ----------------
        for b in range(NCORES):
            for n in range(2):
                rhs = rhspool.tile([P, 8 * 512], dt.bfloat16, tag="rhs")
                if no_cc:
                    src = bounce[:, n * 512:(n + 1) * 512]
                else:
                    src = gathered[b, :, n * 512:(n + 1) * 512]  # [1024, 512]
                nc.sync.dma_start(
                    rhs.rearrange("p (k j) -> p k j", k=8),
                    src.rearrange("(k p) j -> p k j", p=P))
                for m in range(8):
                    ps = pspool.tile([P, 512], dt.float32, tag="ps")
                    for k in range(8):
                        nc.tensor.matmul(
                            ps[:],
                            lhsT=zT[:, k * RLOC + m * P: k * RLOC + (m + 1) * P],
                            rhs=rhs[:, k * 512:(k + 1) * 512],
                            start=(k == 0), stop=(k == 7))
                    scr = scrpool.tile([P, 512], dt.bfloat16, tag="scr")
                    col = m * 16 + b * 2 + n
                    nc.scalar.activation(scr[:], ps[:], AF.Exp, scale=2.0,
                                         accum_out=rs[:, col:col + 1])

        # ---- Phase E: reduce partials, write outputs -------------------
        for m in range(8):
            nc.vector.reduce_sum(rowsum_sb[:, m:m + 1],
                                 rs[:, m * 16:(m + 1) * 16], axis=AX.X)
        nc.sync.dma_start(rowsum_o[:, :], rowsum_sb[:])
        nc.sync.dma_start(posv_o[:, :], pos_sb[:])
        nc.sync.dma_start(diag_o[:, :], diag_sb[:])

    nc.compile()
    return nc


def _get_nc():
    if "nc" not in _CACHE:
        _CACHE["nc"] = _build()
    return _CACHE["nc"]


def kernel(emb_i: np.ndarray, emb_j: np.ndarray) -> np.ndarray:
    from concourse.bass_utils import run_bass_kernel_spmd

    emb_i = np.ascontiguousarray(np.asarray(emb_i, dtype=np.float32))
    emb_j = np.ascontiguousarray(np.asarray(emb_j, dtype=np.float32))
    assert emb_i.shape == (N, D) and emb_j.shape == (N, D)

    ident = np.eye(P, dtype=ml_dtypes.bfloat16)
    in_maps = []
    for c in range(NCORES):
        shard = np.concatenate(
            [emb_i[c * HALF:(c + 1) * HALF], emb_j[c * HALF:(c + 1) * HALF]],
            axis=0)
        in_maps.append({"x": np.ascontiguousarray(shard), "ident": ident})

    nc = _get_nc()
    res = run_bass_kernel_spmd(nc, in_maps, core_ids=list(range(NCORES)))

    total = 0.0
    for c in range(NCORES):
        r = res.results[c]
        rowsum = r["rowsum"].astype(np.float64)  # [128, 8]; local row = m*128+p
        diagv = r["diagv"].astype(np.float64)
        posv = r["posv"].astype(np.float64)      # [128, 4]; pair row = t*128+p
        denom = rowsum - np.exp(2.0 * diagv)
        pos_pm = np.concatenate([posv, posv], axis=1)  # [128, 8]
        total += np.sum(np.log(denom) - 2.0 * pos_pm)
    loss = total / (2.0 * N)
    return np.array(loss, dtype=np.float32)
